# revision 16
# baseline (speedup 1.0000x reference)
"""Trainium2 Bass kernel for nn_DualAttention (sparse_attention).

Algorithm notes
---------------
The reference gathers per-pair mention blocks hfo/tfo = mention_embed[b, h/t]
([N,16,768]) and projects them per pair. But the projections depend only on
the (batch, entity) index, of which there are just B*E = 168, so we compute
relu(X @ W.T) per *entity* (24x less matmul work), then combine per pair:

  s[n,i,j] = hc[eh][i] + qv[et][j] + tq[et][i] * hf16[eh][i,j]   (+ masks)
  h_weight = softmax_i(max_j s);  start_re = h_weight @ hf[eh]
  t_weight = softmax_j(max_i s);  end_re   = t_weight @ tf[et]

Distribution over 8 cores: pairs are sorted by head entity (and separately by
tail entity); core k takes sorted block k of 512 pairs, so its pairs touch a
narrow contiguous band of entities. Each core projects only its band rows,
computes small per-entity tables for its band ([hc'|hf16] interleaved 17 per
mention; [qv'|tq] 2 per mention), and ONE AllGather shares them. The
mention-count NEG masks are folded into hc'/qv' at staging (a masked
mention's score is ~-1e9 either way). Lookups of a pair's own-sort-side
entity hit a LOCAL table copy; only the other side needs the gathered copy.

Per-pair gathers are one-hot matmuls; the final weighted sums are banded
matmuls G.T @ band with G built on-device. The softmax 1/sum is folded into
the weights before G, so outputs DMA straight from PSUM-copy. Weights/tables
move as bf16 (tolerance 2e-2, measured ~7e-3); accumulation is fp32 PSUM.
"""

import numpy as np
import ml_dtypes

import concourse.bass as bass
import concourse.mybir as mybir
import concourse.tile as tile
from concourse.bass_utils import run_bass_kernel_spmd

H = 768
B, E, M = 4, 42, 16
NENT = B * E            # 168
N = 4096
NC = 8
PPC = N // NC           # 512 pairs per core
MT = PPC // 128         # 4 m-tiles of pairs per core
KT = H // 128           # 6 k-tiles over hidden dim
NEG = -1e9

F32 = mybir.dt.float32
F32R = mybir.dt.float32r
BF16 = mybir.dt.bfloat16
NPBF = ml_dtypes.bfloat16


def _split_multi_waits(nc, max_waits=1):
    """walrus codegen in this container rejects >1 sync wait per instruction.

    Move extra waits onto pure-wait EventSemaphore instructions inserted just
    before, on the same engine (engine queues are serial, so ordering and
    semantics are preserved)."""
    for fn in nc.m.functions:
        for bb in fn.blocks:
            new = []
            changed = False
            for ins in bb.instructions:
                si = ins.sync_info
                if si is not None and si.on_wait and len(si.on_wait) > max_waits:
                    waits = list(si.on_wait)
                    for i, w in enumerate(waits[:-max_waits]):
                        ev = mybir.InstEventSemaphore(
                            name=f"{ins.name}-xw{i}", engine=ins.engine
                        )
                        ev.sync_info = mybir.SyncInfo(on_wait=[w], on_update=[])
                        ev.debug = ins.debug
                        new.append(ev)
                    si.on_wait = waits[-max_waits:]
                    changed = True
                new.append(ins)
            if changed:
                bb.instructions = new


def _band(ent_sorted):
    lo_row = 16 * int(ent_sorted.min())
    hi_row = 16 * int(ent_sorted.max()) + 16
    lo_tile = lo_row // 128
    nb = (hi_row - 128 * lo_tile + 127) // 128
    return lo_tile, nb


def _kmajor(a):
    """[KT*128, W] f32 -> [128, KT*W] bf16, k-tiles along columns."""
    W = a.shape[1]
    out = np.empty((128, KT * W), np.float32)
    for kt in range(KT):
        out[:, kt * W:(kt + 1) * W] = a[kt * 128:(kt + 1) * 128]
    return np.ascontiguousarray(out.astype(NPBF))


def _prep(inputs):
    f32 = np.float32
    mention = np.ascontiguousarray(inputs["mention_embed"], dtype=f32)
    mention_flat = mention.reshape(NENT * M, H)
    b_ind = np.asarray(inputs["b_ind"]).astype(np.int64)
    h_ind = np.asarray(inputs["h_ind"]).astype(np.int64)
    t_ind = np.asarray(inputs["t_ind"]).astype(np.int64)
    mention_num = np.asarray(inputs["mention_num"]).astype(np.int64)

    eh = (b_ind * E + h_ind).astype(np.int64)
    et = (b_ind * E + t_ind).astype(np.int64)
    mnum_flat = mention_num.reshape(NENT)

    h_order = np.argsort(eh, kind="stable")
    t_order = np.argsort(et, kind="stable")

    lo_h, nb_h, lo_t, nb_t = [], [], [], []
    for k in range(NC):
        lo, nb = _band(eh[h_order[k * PPC:(k + 1) * PPC]])
        lo_h.append(lo); nb_h.append(nb)
        lo, nb = _band(et[t_order[k * PPC:(k + 1) * PPC]])
        lo_t.append(lo); nb_t.append(nb)
    NBH = max(nb_h)
    NBT = max(nb_t)

    def slots_for(nb):
        need = nb * 8
        for s in (16, 32, 64, 128):
            if need <= s:
                return s
        raise ValueError(f"band too wide: {nb} tiles")
    SLH = slots_for(NBH)
    SLT = slots_for(NBT)
    TH = NC * SLH // 128
    TT = NC * SLT // 128
    assert NC * SLH <= 256 and NC * SLT <= 256  # bf16-exact one-hot values

    def table_rows(lo_list, nb, slots):
        rows = np.full(NENT, -1, np.int64)
        for k in reversed(range(NC)):
            base_ent = 8 * lo_list[k]
            ents = np.arange(base_ent, min(base_ent + nb * 8, NENT))
            rows[ents] = k * slots + (ents - base_ent)
        return rows
    hrow = table_rows(lo_h, NBH, SLH)
    trow = table_rows(lo_t, NBT, SLT)

    # band-tile -> pair-m-tile ranges (union over cores) for the G matmuls
    rngH = [[MT, 0] for _ in range(NBH)]
    rngT = [[MT, 0] for _ in range(NBT)]
    per_core = []
    for k in range(NC):
        ph = h_order[k * PPC:(k + 1) * PPC]
        pt = t_order[k * PPC:(k + 1) * PPC]
        ehh, eth = eh[ph], et[ph]
        eht, ett = eh[pt], et[pt]
        lrow_h = ehh - 8 * lo_h[k]          # local slot of h-ordered pairs
        lrow_t = ett - 8 * lo_t[k]

        for rng, lrow in ((rngH, lrow_h), (rngT, lrow_t)):
            for mt in range(MT):
                sl = lrow[mt * 128:(mt + 1) * 128]
                for kt in range(int(sl.min()) // 8, int(sl.max()) // 8 + 1):
                    rng[kt][0] = min(rng[kt][0], mt)
                    rng[kt][1] = max(rng[kt][1], mt + 1)

        def xt_for(lo, nb):
            rows = np.zeros((nb * 128, H), f32)
            g0 = 128 * lo
            g1 = min(g0 + nb * 128, NENT * M)
            rows[: g1 - g0] = mention_flat[g0:g1]
            return _kmajor(np.ascontiguousarray(rows.T))  # [128, KT*nb*128]
        xt_h = xt_for(lo_h[k], NBH)
        xt_t = xt_for(lo_t[k], NBT)

        vals = np.concatenate([
            lrow_h, trow[eth], hrow[eht], lrow_t,
        ]).astype(NPBF)[None, :]                          # [1, 4*512]

        pp = np.arange(128)
        def maskcols(lo, nb):
            cols = []
            for c in range(nb):
                ent = (128 * lo + 128 * c + pp) // 16
                i = pp % 16
                bad = (ent >= NENT) | (i >= mnum_flat[np.minimum(ent, NENT - 1)])
                cols.append(np.where(bad, NEG, 0.0).astype(f32))
            return cols
        maskb = np.stack(maskcols(lo_h[k], NBH) + maskcols(lo_t[k], NBT),
                         axis=1)                          # [128, NBH+NBT]

        per_core.append(dict(xt_h=xt_h, xt_t=xt_t, vals=vals, maskb=maskb))

    wts = np.concatenate(
        [_kmajor(np.ascontiguousarray(np.asarray(inputs["W_head"], f32).T)),
         _kmajor(np.ascontiguousarray(np.asarray(inputs["W_tail"], f32).T))],
        axis=1)                                           # [128, 2*KT*768]
    wvec = np.concatenate([
        np.tile(np.asarray(inputs["w_c"], f32)[None, :], (128, 1)),
        np.tile(np.asarray(inputs["w_q"], f32)[None, :], (128, 1)),
        np.tile(np.asarray(inputs["w_cq"], f32)[None, :], (128, 1)),
    ], axis=1).astype(NPBF)                               # [128, 2304] bf16

    NBMX = max(NBH, NBT)
    TMX = max(TH, TT)
    entloc = np.stack([((128 * c + np.arange(128)) // 16).astype(f32)
                       for c in range(NBMX)], axis=1)     # [128, NBMX]
    iotaP = np.stack([(128 * c + np.arange(128)).astype(f32)
                      for c in range(TMX)], axis=1)       # [128, TMX]
    repm = (np.arange(16)[:, None] == (np.arange(128) % 16)[None, :]).astype(f32)

    meta = dict(NBH=NBH, NBT=NBT, SLH=SLH, SLT=SLT, TH=TH, TT=TT,
                rngH=tuple(map(tuple, rngH)), rngT=tuple(map(tuple, rngT)))
    shared = dict(wts=wts, wvec=wvec, entloc=entloc, iotaP=iotaP,
                  identity=np.eye(128, dtype=np.float32), repm=repm)
    post = dict(h_order=h_order, t_order=t_order)
    return meta, shared, per_core, post


def _build(meta, sim_single=False):
    NBH, NBT = meta["NBH"], meta["NBT"]
    SLH, SLT = meta["SLH"], meta["SLT"]
    TH, TT = meta["TH"], meta["TT"]
    rngH, rngT = meta["rngH"], meta["rngT"]
    NBMX = max(NBH, NBT)
    TMX = max(TH, TT)
    # staging shard: H records (272/slot: [hc_i|hf16_i,:] x 16) then T records
    SECH = 0
    SECT = SLH * 272
    SHARD = SLH * 272 + SLT * 32
    assert SHARD % 128 == 0

    nc = bass.Bass("TRN2", num_devices=(1 if sim_single else NC))
    xt_h = nc.dram_tensor("xt_h", [128, KT * NBH * 128], BF16, kind="ExternalInput")
    xt_t = nc.dram_tensor("xt_t", [128, KT * NBT * 128], BF16, kind="ExternalInput")
    wts = nc.dram_tensor("wts", [128, KT * 2 * H], BF16, kind="ExternalInput")
    wvec = nc.dram_tensor("wvec", [128, 3 * H], BF16, kind="ExternalInput")
    vals = nc.dram_tensor("vals", [1, 4 * PPC], BF16, kind="ExternalInput")
    entloc = nc.dram_tensor("entloc", [128, NBMX], F32, kind="ExternalInput")
    iotaP = nc.dram_tensor("iotaP", [128, TMX], F32, kind="ExternalInput")
    maskb = nc.dram_tensor("maskb", [128, NBH + NBT], F32, kind="ExternalInput")
    identity = nc.dram_tensor("identity", [128, 128], F32, kind="ExternalInput")
    repm = nc.dram_tensor("repm", [16, 128], F32R, kind="ExternalInput")
    reh = nc.dram_tensor("reh", [PPC, H], BF16, kind="ExternalOutput")
    ret = nc.dram_tensor("ret", [PPC, H], BF16, kind="ExternalOutput")

    with tile.TileContext(nc, num_cores=NC) as tc:
        with (
            tc.tile_pool(name="const", bufs=1) as cpool,
            tc.tile_pool(name="band", bufs=1) as bpool,
            tc.tile_pool(name="work", bufs=3) as wpool,
            tc.tile_pool(name="keep", bufs=1) as gpool,
            tc.tile_pool(name="small", bufs=4) as spool,
            # "proj" big psum: [128,768] = 2 banks x 2 bufs; "sm"+"wrep"
            # small psum: 1 bank x (3+1) -> 8 banks total = PSUM capacity
            tc.tile_pool(name="psum", bufs=2, space="PSUM") as ppool,
            tc.tile_pool(name="psg", bufs=3, space="PSUM") as pgpool,
            tc.tile_pool(name="psw", bufs=1, space="PSUM") as pwpool,
            tc.tile_pool(name="dram", bufs=1, space="DRAM") as dpool,
        ):
            # ---- batched input DMAs (k-major packed on host) ----
            XH = bpool.tile([128, KT * NBH * 128], BF16, tag="XH")
            XT = bpool.tile([128, KT * NBT * 128], BF16, tag="XT")
            W3 = cpool.tile([128, KT * 2 * H], BF16, tag="W3")
            hw_h = KT * NBH * 128 // 2
            hw_t = KT * NBT * 128 // 2
            hw_w = KT * H
            nc.sync.dma_start(XH[:, :hw_h], xt_h.ap()[:, :hw_h])
            nc.sync.dma_start(XH[:, hw_h:], xt_h.ap()[:, hw_h:])
            nc.sync.dma_start(W3[:, :hw_w], wts.ap()[:, :hw_w])
            nc.sync.dma_start(W3[:, hw_w:], wts.ap()[:, hw_w:])
            nc.sync.dma_start(XT[:, :hw_t], xt_t.ap()[:, :hw_t])
            nc.sync.dma_start(XT[:, hw_t:], xt_t.ap()[:, hw_t:])

            def xh_k(kt):
                return XH[:, kt * NBH * 128:(kt + 1) * NBH * 128]
            def xt_k(kt):
                return XT[:, kt * NBT * 128:(kt + 1) * NBT * 128]
            def w_k(kt, c0, c1):
                # wts is [head k-major | tail k-major]
                base = (0 if c0 < H else KT * H)
                cc0 = c0 - (0 if c0 < H else H)
                cc1 = c1 - (0 if c0 < H else H)
                return W3[:, base + kt * H + cc0: base + kt * H + cc1]

            wvec_sb = cpool.tile([128, 3 * H], BF16)
            nc.sync.dma_start(wvec_sb[:], wvec.ap())
            entloc_sb = cpool.tile([128, NBMX], F32)
            nc.sync.dma_start(entloc_sb[:], entloc.ap())
            iotaP_sb = cpool.tile([128, TMX], F32)
            nc.sync.dma_start(iotaP_sb[:], iotaP.ap())
            maskb_sb = cpool.tile([128, NBH + NBT], F32)
            nc.sync.dma_start(maskb_sb[:], maskb.ap())
            ident = cpool.tile([128, 128], F32)
            nc.sync.dma_start(ident[:], identity.ap())
            repm_sb = cpool.tile([16, 128], F32R)
            nc.sync.dma_start(repm_sb[:], repm.ap())
            vrep = cpool.tile([128, 4 * PPC], BF16)
            nc.sync.dma_start(
                vrep[:],
                bass.AP(tensor=vals.ap().tensor, offset=0,
                        ap=[[0, 128], [1, 4 * PPC]]))

            stage = dpool.tile([SHARD], BF16)
            cc = dpool.tile(
                [NC * SHARD], BF16,
                **({} if sim_single else {"addr_space": "Shared"}))
            zero_sb = cpool.tile([128, SHARD // 128], BF16)
            nc.vector.memset(zero_sb, 0.0)
            nc.scalar.dma_start(
                stage[:].rearrange("(p c) -> p c", p=128), zero_sb[:])

            # ---- phase A: projections + rowdots + staging ----
            def project(xk, nb, woff, tag):
                out = []
                for mt in range(nb):
                    psA = ppool.tile([128, 512], F32, space="PSUM", tag="projA")
                    psB = ppool.tile([128, 256], F32, space="PSUM", tag="projB")
                    for ps, w0, w1 in ((psA, 0, 512), (psB, 512, 768)):
                        for kt in range(KT):
                            nc.tensor.matmul(
                                ps[:, 0:w1 - w0],
                                lhsT=xk(kt)[:, mt * 128:(mt + 1) * 128],
                                rhs=w_k(kt, woff + w0, woff + w1),
                                start=(kt == 0), stop=(kt == KT - 1))
                    t = bpool.tile([128, H], F32R, tag=f"{tag}{mt}")
                    nc.scalar.activation(t[:, 0:512], psA[:],
                                         mybir.ActivationFunctionType.Relu)
                    nc.vector.tensor_scalar_max(t[:, 512:768], psB[:], 0.0)
                    out.append(t)
                return out

            def rowdot(src_tile, wcol, acc_ap, eng):
                prod = wpool.tile([128, H], BF16, tag="prod")
                if eng is nc.vector:
                    eng.scalar_tensor_tensor(
                        out=prod[:], in0=src_tile[:].bitcast(F32), scalar=1.0,
                        in1=wvec_sb[:, wcol * H:(wcol + 1) * H],
                        op0=mybir.AluOpType.mult, op1=mybir.AluOpType.mult,
                        accum_out=acc_ap)
                else:
                    eng.tensor_tensor(
                        out=prod[:], in0=src_tile[:].bitcast(F32),
                        in1=wvec_sb[:, wcol * H:(wcol + 1) * H],
                        op=mybir.AluOpType.mult)
                    nc.scalar.activation(
                        prod[:], prod[:], mybir.ActivationFunctionType.Copy,
                        accum_out=acc_ap)

            hfb = project(xh_k, NBH, 0, "hfb")
            for mt in range(NBH):
                acc = spool.tile([128, 1], F32, tag="accH")
                rowdot(hfb[mt], 0, acc[:],
                       nc.vector if mt % 2 == 0 else nc.gpsimd)
                stH = spool.tile([128, 17], BF16, tag="stH")
                nc.vector.tensor_tensor(
                    out=stH[:, 0:1], in0=acc[:],
                    in1=maskb_sb[:, mt:mt + 1], op=mybir.AluOpType.add)
                nc.scalar.activation(stH[:, 1:17], hfb[mt][:, :16].bitcast(F32),
                                     mybir.ActivationFunctionType.Copy)
                nc.sync.dma_start(
                    bass.AP(tensor=stage[:].tensor, offset=SECH + 2176 * mt,
                            ap=[[17, 128], [1, 17]]), stH[:])

            tfb = project(xt_k, NBT, H, "tfb")
            for mt in range(NBT):
                accq = spool.tile([128, 1], F32, tag="accQ")
                acct = spool.tile([128, 1], F32, tag="accT")
                rowdot(tfb[mt], 1, accq[:], nc.vector)
                rowdot(tfb[mt], 2, acct[:], nc.vector)
                stT = spool.tile([128, 2], BF16, tag="stT")
                nc.vector.tensor_tensor(
                    out=stT[:, 0:1], in0=accq[:],
                    in1=maskb_sb[:, NBH + mt:NBH + mt + 1],
                    op=mybir.AluOpType.add)
                nc.scalar.activation(stT[:, 1:2], acct[:],
                                     mybir.ActivationFunctionType.Copy)
                nc.sync.dma_start(
                    bass.AP(tensor=stage[:].tensor, offset=SECT + 256 * mt,
                            ap=[[2, 128], [1, 2]]), stT[:])

            # ---- local tables (own band; no collective dependency) ----
            Hloc = cpool.tile([SLH, 272], BF16, tag="Hloc")
            nc.sync.dma_start(
                Hloc[:], bass.AP(tensor=stage[:].tensor, offset=SECH,
                                 ap=[[272, SLH], [1, 272]]))
            Tloc = cpool.tile([SLT, 32], BF16, tag="Tloc")
            nc.sync.dma_start(
                Tloc[:], bass.AP(tensor=stage[:].tensor, offset=SECT,
                                 ap=[[32, SLT], [1, 32]]))

            # ---- one-hots (no collective dependency) ----
            def build_oh(vcol, ktiles, rows, tag):
                tiles = []
                for kt in range(ktiles):
                    t = bpool.tile([rows, PPC], BF16, tag=f"{tag}{kt}")
                    nc.gpsimd.tensor_scalar(
                        out=t[:],
                        in0=vrep[0:rows, vcol * PPC:(vcol + 1) * PPC],
                        scalar1=iotaP_sb[0:rows, kt:kt + 1], scalar2=None,
                        op0=mybir.AluOpType.is_equal)
                    tiles.append(t)
                return tiles
            ohHhL = build_oh(0, 1, SLH, "ohHhL")[0]
            ohThG = build_oh(1, TT, 128, "ohThG")
            ohHtG = build_oh(2, TH, 128, "ohHtG")
            ohTtL = build_oh(3, 1, SLT, "ohTtL")[0]

            # ---- ONE AllGather ----
            if sim_single:
                nc.sync.dma_start(cc[0:SHARD], stage[:])
                nc.sync.dma_start(cc[(NC - 1) * SHARD: NC * SHARD], stage[:])
            else:
                nc.gpsimd.collective_compute(
                    "AllGather", mybir.AluOpType.bypass,
                    replica_groups=[list(range(NC))],
                    ins=[stage.opt()], outs=[cc.opt()],
                )

            # ---- gathered tables (HWDGE, one DMA each) ----
            def tbl_src(sec, rec, tt, slots):
                cpt = 128 // slots
                return bass.AP(
                    tensor=cc[:].tensor, offset=tt * cpt * SHARD + sec,
                    ap=[[SHARD, cpt], [rec, slots], [1, rec]])
            Htab = []
            for tt in range(TH):
                t = cpool.tile([128, 272], BF16, tag=f"htab{tt}")
                nc.sync.dma_start(t[:], tbl_src(SECH, 272, tt, SLH))
                Htab.append(t)
            Ttab = []
            for tt in range(TT):
                t = cpool.tile([128, 32], BF16, tag=f"ttab{tt}")
                nc.sync.dma_start(t[:], tbl_src(SECT, 32, tt, SLT))
                Ttab.append(t)

            # ---- phase B (both orders) then phase C (both orders) ----
            def phase_b(h_side):
                ez2s = []
                for mt in range(MT):
                    msl = slice(mt * 128, (mt + 1) * 128)
                    HT = pgpool.tile([128, 304], F32, space="PSUM", tag="sm")
                    Hg, Tg = HT[:, 0:272], HT[:, 272:304]
                    if h_side:
                        nc.tensor.matmul(Hg, lhsT=ohHhL[:, msl], rhs=Hloc[:],
                                         start=True, stop=True)
                        for kt in range(TT):
                            nc.tensor.matmul(
                                Tg, lhsT=ohThG[kt][:, msl], rhs=Ttab[kt][:],
                                start=(kt == 0), stop=(kt == TT - 1))
                    else:
                        for kt in range(TH):
                            nc.tensor.matmul(
                                Hg, lhsT=ohHtG[kt][:, msl], rhs=Htab[kt][:],
                                start=(kt == 0), stop=(kt == TH - 1))
                        nc.tensor.matmul(Tg, lhsT=ohTtL[:, msl], rhs=Tloc[:],
                                         start=True, stop=True)
                    HgR = Hg.rearrange("p (i k) -> p i k", k=17)
                    Tgs = wpool.tile([128, 32], F32, tag="Tgs")
                    nc.scalar.activation(Tgs[:], Tg,
                                         mybir.ActivationFunctionType.Copy)
                    hcs = wpool.tile([128, 16], F32, tag="hcs")
                    nc.scalar.activation(
                        hcs[:], HgR[:, :, 0:1].rearrange("p i k -> p (i k)"),
                        mybir.ActivationFunctionType.Copy)
                    tq_b = Tgs[:].rearrange("p (j k) -> p j k", k=2)[:, :, 1:2]
                    qv_b = Tgs[:].rearrange("p (j k) -> p k j", k=2)[:, 0:1, :]

                    # s = tq[i]*hf16[i,j] + (hc'[i] (+) qv'[j])
                    # (DVE reads the PSUM operand; GPSIMD cannot touch PSUM)
                    aeng = nc.gpsimd if mt % 2 else nc.vector
                    ueng = nc.vector if mt % 2 else nc.gpsimd
                    s = wpool.tile([128, 16, 16], F32, tag="s")
                    nc.vector.tensor_tensor(
                        out=s[:], in0=tq_b.to_broadcast((128, 16, 16)),
                        in1=HgR[:, :, 1:17], op=mybir.AluOpType.mult)
                    u = wpool.tile([128, 16, 16], F32, tag="u")
                    ueng.tensor_tensor(
                        out=u[:],
                        in0=hcs[:, :, None].to_broadcast((128, 16, 16)),
                        in1=qv_b.to_broadcast((128, 16, 16)),
                        op=mybir.AluOpType.add)
                    aeng.tensor_tensor(out=s[:], in0=s[:], in1=u[:],
                                       op=mybir.AluOpType.add)
                    red = spool.tile([128, 16], F32, tag="red")
                    if h_side:
                        nc.vector.tensor_reduce(
                            out=red[:], in_=s[:], axis=mybir.AxisListType.X,
                            op=mybir.AluOpType.max)
                    else:
                        nc.vector.tensor_reduce(
                            out=red[:], in_=s[:].rearrange("p i j -> p j i"),
                            axis=mybir.AxisListType.X, op=mybir.AluOpType.max)
                    nm1 = spool.tile([128, 1], F32, tag="nm1")
                    nc.vector.tensor_reduce(out=nm1[:], in_=red[:],
                                            axis=mybir.AxisListType.X,
                                            op=mybir.AluOpType.max, negate=True)
                    ez = spool.tile([128, 16], F32, tag="ez")
                    ssum = spool.tile([128, 1], F32, tag="ssum")
                    nc.scalar.activation(ez[:], red[:],
                                         mybir.ActivationFunctionType.Exp,
                                         bias=nm1[:], scale=1.0,
                                         accum_out=ssum[:])
                    rs = spool.tile([128, 1], F32, tag="rs")
                    nc.vector.reciprocal(rs[:], ssum[:])
                    # fold softmax 1/sum into the weights
                    ez2 = gpool.tile([128, 16], F32, tag=f"ez{h_side}{mt}")
                    nc.vector.tensor_scalar_mul(ez2[:], ez[:], rs[:])
                    ez2s.append(ez2)
                return ez2s

            def phase_c(h_side, band, nb, rng, ez2s, out_dram):
                vcol = 0 if h_side else 3
                # C: transposes -> wT -> wrep -> G -> banded matmuls
                tpB = pgpool.tile([16, PPC], F32, space="PSUM", tag="sm")
                for mt in range(MT):
                    nc.tensor.transpose(tpB[:, mt * 128:(mt + 1) * 128],
                                        ez2s[mt][:], ident[:])
                wT = gpool.tile([16, PPC], F32R, tag="wT")
                nc.vector.tensor_copy(wT[:], tpB[:])
                wrep = pwpool.tile([128, PPC], F32, space="PSUM", tag="wrep")
                nc.tensor.matmul(wrep[:], lhsT=repm_sb[:], rhs=wT[:],
                                 start=True, stop=True)
                wrep_sb = gpool.tile([128, PPC], F32, tag="wrep_sb")
                nc.scalar.activation(wrep_sb[:], wrep[:],
                                     mybir.ActivationFunctionType.Copy)
                gts = []
                for kt in range(nb):
                    c0, c1 = rng[kt][0] * 128, rng[kt][1] * 128
                    gt = gpool.tile([128, c1 - c0], F32R,
                                    tag=f"gt{h_side}{kt}")
                    nc.vector.scalar_tensor_tensor(
                        out=gt[:],
                        in0=vrep[:, vcol * PPC + c0: vcol * PPC + c1],
                        scalar=entloc_sb[:, kt:kt + 1],
                        in1=wrep_sb[:, c0:c1],
                        op0=mybir.AluOpType.is_equal,
                        op1=mybir.AluOpType.mult)
                    gts.append(gt)
                for mt in range(MT):
                    kts = [kt for kt in range(nb)
                           if rng[kt][0] <= mt < rng[kt][1]]
                    psA = ppool.tile([128, 512], F32, space="PSUM", tag="projA")
                    psB = ppool.tile([128, 256], F32, space="PSUM", tag="projB")
                    for ps, w0, w1 in ((psA, 0, 512), (psB, 512, 768)):
                        for i, kt in enumerate(kts):
                            c0 = rng[kt][0] * 128
                            nc.tensor.matmul(
                                ps[:, 0:w1 - w0],
                                lhsT=gts[kt][:, mt * 128 - c0:
                                             (mt + 1) * 128 - c0],
                                rhs=band[kt][:, w0:w1],
                                start=(i == 0), stop=(i == len(kts) - 1))
                    o = wpool.tile([128, H], BF16, tag="o")
                    nc.scalar.activation(o[:, 0:512], psA[:],
                                         mybir.ActivationFunctionType.Copy)
                    nc.vector.tensor_copy(o[:, 512:768], psB[:])
                    if mt % 2 == 0:
                        nc.sync.dma_start(
                            out_dram.ap()[mt * 128:(mt + 1) * 128, :], o[:])
                    else:
                        nc.gpsimd.dma_start(
                            out_dram.ap()[mt * 128:(mt + 1) * 128, :], o[:])

            ezh = phase_b(True)
            ezt = phase_b(False)
            phase_c(True, hfb, NBH, rngH, ezh, reh)
            phase_c(False, tfb, NBT, rngT, ezt, ret)

    _split_multi_waits(nc)
    return nc


_CACHE = {}
_PREP_CACHE = {}

_IN_NAMES = ("xt_h", "xt_t", "wts", "wvec", "vals", "entloc", "iotaP",
             "maskb", "identity", "repm")


def kernel(**inputs):
    pkey = hash(tuple(
        np.asarray(inputs[n]).tobytes()
        for n in ("mention_embed", "b_ind", "h_ind", "t_ind", "mention_num",
                  "W_head", "W_tail", "w_c", "w_q", "w_cq")))
    if pkey not in _PREP_CACHE:
        _PREP_CACHE.clear()
        _PREP_CACHE[pkey] = _prep(inputs)
    meta, shared, per_core, post = _PREP_CACHE[pkey]
    key = tuple(sorted(meta.items()))
    if key not in _CACHE:
        _CACHE[key] = _build(meta)
    nc = _CACHE[key]

    in_maps = []
    for k in range(NC):
        m = dict(per_core[k])
        m.update(shared)
        in_maps.append({name: np.ascontiguousarray(m[name])
                        for name in _IN_NAMES})

    res = run_bass_kernel_spmd(nc, in_maps, list(range(NC)))

    start_re = np.empty((N, H), np.float32)
    end_re = np.empty((N, H), np.float32)
    h_order, t_order = post["h_order"], post["t_order"]
    for k in range(NC):
        start_re[h_order[k * PPC:(k + 1) * PPC]] = \
            np.asarray(res.results[k]["reh"]).astype(np.float32)
        end_re[t_order[k * PPC:(k + 1) * PPC]] = \
            np.asarray(res.results[k]["ret"]).astype(np.float32)

    entity = np.asarray(inputs["entity_embed"], np.float32)
    b_ind = np.asarray(inputs["b_ind"]).astype(np.int64)
    h_ind = np.asarray(inputs["h_ind"]).astype(np.int64)
    t_ind = np.asarray(inputs["t_ind"]).astype(np.int64)
    head_embed = np.concatenate([entity[b_ind, h_ind], start_re], axis=-1)
    tail_embed = np.concatenate([entity[b_ind, t_ind], end_re], axis=-1)
    return head_embed, tail_embed
